# revision 1
# baseline (speedup 1.0000x reference)
"""BiLSTM Trainium2 kernel.

Problem: B=32, T=512, I=512, H=512 bidirectional LSTM (torch gate order
i,f,g,o; shared Wx/Wh/bx/bh across directions; backward outputs stacked in
processing order, i.e. out[:, t, H:] is the backward cell's state after
processing x[:, T-1-t]).

Sharding: 8 cores = 2 directions x 4 batch groups of 8. Every core runs the
IDENTICAL forward-LSTM program; backward cores receive their x time-reversed
on the host, which makes the program SPMD and the output assembly flip-free.

Per-core device program (one direction, B_l=8):
  - The recurrent matmul h @ Wh.T keeps h stationary in the PE (lhsT
    [K=128, M=8] slices of hT) and streams WhT as float32r (1 cycle/row).
  - gx = x @ WxT (+ biases) is computed on-chip in 16-step windows,
    interleaved into the PE bubbles of the recurrence, so there is no
    gx DRAM round trip and the PE never idles long enough to re-throttle.
  - Gates land in four per-gate PSUM tiles [8, 512] (host-permuted order
    i,f,o,g) so each gate's activation can start the moment its 4
    accumulating matmuls finish, overlapping the rest of the PE stream.
  - The epilogue is half-chunked and ends in the transposed domain:
    hT = transpose(sigmoid_o) * transpose(tanh(c)) via PE-transposes plus a
    [128, 16] DVE multiply per half, so the next step's matmul stream starts
    as soon as the first half of hT exists. y is stored transposed and
    un-transposed on the host.
"""

import numpy as np

B, T, I, H = 32, 512, 512, 512
G4 = 4 * H            # 2048 gate width
BL = 8                # batch rows per core
WIN = 16              # steps per gx window (WIN * BL = 128 rows)
NW = T // WIN         # number of windows

_COMPILED = {}


def _build_program(t_steps: int):
    import concourse.bass as bass
    import concourse.tile as tile
    from concourse import bacc, mybir

    dt = mybir.dt
    f32 = dt.float32
    f32r = dt.float32r
    nw = t_steps // WIN

    nc = bacc.Bacc("TRN2", target_bir_lowering=False, debug=False)

    xT = nc.declare_dram_parameter("xT", [I, t_steps * BL], f32r, isOutput=False)
    WxT_d = nc.declare_dram_parameter("WxT", [I, G4], f32r, isOutput=False)
    WhT_d = nc.declare_dram_parameter("WhT", [H, G4], f32r, isOutput=False)
    b128_d = nc.declare_dram_parameter("b128", [128, G4], f32, isOutput=False)
    eye_d = nc.declare_dram_parameter("eye", [128, 128], f32r, isOutput=False)
    z_d = nc.declare_dram_parameter("z", [128, 4 * BL], f32r, isOutput=False)
    eye32_d = nc.declare_dram_parameter("eye32", [BL, BL], f32, isOutput=False)
    y_d = nc.declare_dram_parameter("y", [t_steps, 128, 4 * BL], f32r, isOutput=True)

    with tile.TileContext(nc) as tc:
        with (
            tc.tile_pool(name="const", bufs=1) as const_pool,
            tc.tile_pool(name="xT", bufs=8) as xT_pool,
            tc.tile_pool(name="gx", bufs=2) as gx_pool,
            tc.tile_pool(name="ep", bufs=2) as ep_pool,
            tc.tile_pool(name="hT", bufs=2) as hT_pool,
            tc.tile_pool(name="gates", bufs=1, space="PSUM") as gates_pool,
            tc.tile_pool(name="gxps", bufs=1, space="PSUM") as gxps_pool,
            tc.tile_pool(name="trps", bufs=1, space="PSUM") as trps_pool,
        ):
            # ---- constants ----
            whT = []
            for k in range(4):
                t_ = const_pool.tile([128, G4], f32r, tag=f"whT{k}", name=f"whT{k}")
                nc.sync.dma_start(out=t_, in_=WhT_d[k * 128 : (k + 1) * 128, :])
                whT.append(t_)
            wxT = []
            for k in range(4):
                t_ = const_pool.tile([128, G4], f32r, tag=f"wxT{k}", name=f"wxT{k}")
                nc.sync.dma_start(out=t_, in_=WxT_d[k * 128 : (k + 1) * 128, :])
                wxT.append(t_)
            b128 = const_pool.tile([128, G4], f32, tag="b128")
            nc.sync.dma_start(out=b128, in_=b128_d[:, :])
            eye = const_pool.tile([128, 128], f32r, tag="eye")
            nc.sync.dma_start(out=eye, in_=eye_d[:, :])
            eye32 = const_pool.tile([BL, BL], f32, tag="eye32")
            nc.sync.dma_start(out=eye32, in_=eye32_d[:, :])

            # ---- xT window loads (window w -> 4 tiles [128 I-chunk, 128 rows])
            xT_tiles = {}

            def load_xT(w):
                tiles = []
                for k in range(4):
                    t_ = xT_pool.tile([128, 128], f32r, tag="xT", name=f"xt{w}_{k}")
                    nc.sync.dma_start(
                        out=t_,
                        in_=xT[k * 128 : (k + 1) * 128, w * 128 : (w + 1) * 128],
                    )
                    tiles.append(t_)
                xT_tiles[w] = tiles

            # ---- gx compute for one window, in 4 single-bank parts ----
            # part p in 0..3 computes gate n-chunk p (cols p*512..+512) in a
            # [128, 512] PSUM tile; a DVE add folds the bias in and moves the
            # part to SBUF.
            gx_sb = {}
            gx_ps = {}

            def emit_gx_mms(w, part):
                if part == 0:
                    gx_sb[w] = gx_pool.tile([128, G4], f32r, tag="gx", name=f"gx{w}")
                gx_ps[w] = gxps_pool.tile([128, 512], f32, tag="gxps", name=f"gxps{w}_{part}")
                ps = gx_ps[w]
                xt = xT_tiles[w]
                n0 = part * 512
                for k in range(4):
                    nc.tensor.matmul(
                        ps,
                        lhsT=xt[k],
                        rhs=wxT[k][:, n0 : n0 + 512],
                        start=(k == 0),
                        stop=(k == 3),
                    )

            def emit_gx_add(w, part):
                # fold bias, move the finished PSUM quarter to SBUF
                n0 = part * 512
                nc.vector.tensor_add(
                    gx_sb[w][:, n0 : n0 + 512],
                    gx_ps[w][:, :],
                    b128[:, n0 : n0 + 512],
                )
                if part == 3:
                    del xT_tiles[w]
                del gx_ps[w]

            # ---- prologue ----
            load_xT(0)
            if nw > 1:
                load_xT(1)
            for p in range(4):
                emit_gx_mms(0, p)
                emit_gx_add(0, p)

            hT = hT_pool.tile([128, 4 * BL], f32r, tag="hT")
            nc.sync.dma_start(out=hT, in_=z_d[:, :])
            c = ep_pool.tile([BL, 512], f32, tag="c")
            nc.vector.memset(c, 0.0)

            sigf = mybir.ActivationFunctionType.Sigmoid
            tanhf = mybir.ActivationFunctionType.Tanh

            # gate layout (host-permuted): n0=i, n1=f, n2=o, n3=g
            def nsl(n):
                return slice(n * 512, (n + 1) * 512)

            # ---- main loop ----
            def alloc_gates(t):
                g = [
                    gates_pool.tile([BL, 512], f32, tag=f"gates{n}", name=f"gates{n}_{t}")
                    for n in range(3)
                ]
                g += [
                    gates_pool.tile([BL, 256], f32, tag=f"gates3{h}", name=f"gates3{h}_{t}")
                    for h in ("a", "b")
                ]
                return g

            def emit_selectors(t, gates):
                w, j = t // WIN, t % WIN
                gxbuf = gx_sb[w]
                for n in range(3):
                    nc.tensor.matmul(
                        gates[n],
                        lhsT=eye[:, j * BL : (j + 1) * BL],
                        rhs=gxbuf[:, nsl(n)],
                        start=True,
                        stop=False,
                    )
                for h in (0, 1):
                    nc.tensor.matmul(
                        gates[3 + h],
                        lhsT=eye[:, j * BL : (j + 1) * BL],
                        rhs=gxbuf[:, 1536 + h * 256 : 1536 + (h + 1) * 256],
                        start=True,
                        stop=False,
                    )

            gates = alloc_gates(0)
            emit_selectors(0, gates)

            for t in range(t_steps):
                w, j = t // WIN, t % WIN

                def rec_mm(n, ks, last=False, cols=None):
                    c0, c1 = (0, 512) if cols is None else cols
                    gcol = min(n, 3) * 512
                    for k in ks:
                        nc.tensor.matmul(
                            gates[n],
                            lhsT=hT[:, k * BL : (k + 1) * BL],
                            rhs=whT[k][:, gcol + c0 : gcol + c1],
                            start=False,
                            stop=(last and k == ks[-1]),
                        )

                # PE: recurrent stream. f,i with k0/k1 before k2/k3 so the
                # late-arriving second hT half is never waited on; then the
                # g gate in two 256-col halves (separate PSUM tiles) so
                # tanh_g chunk 0 starts ~450ns earlier; o last.
                rec_mm(1, (0, 1))
                rec_mm(0, (0, 1))
                rec_mm(1, (2, 3), last=True)
                rec_mm(0, (2, 3), last=True)
                rec_mm(3, (0, 1, 2, 3), last=True, cols=(0, 256))
                rec_mm(4, (0, 1, 2, 3), last=True, cols=(256, 512))
                rec_mm(2, (0, 1, 2, 3), last=True)

                # ACT in dependency-arrival order (FIFO)
                tg = ep_pool.tile([BL, 512], f32, tag="tg")
                si = ep_pool.tile([BL, 512], f32, tag="si")
                sf = ep_pool.tile([BL, 512], f32, tag="sf")
                so = ep_pool.tile([BL, 512], f32, tag="so")
                ig = ep_pool.tile([BL, 512], f32, tag="ig")
                fc = ep_pool.tile([BL, 512], f32, tag="fc")
                cn = ep_pool.tile([BL, 512], f32, tag="c")
                tc_t = ep_pool.tile([BL, 512], f32, tag="tanc")

                HF = 256  # tail chunk = half the hidden dim
                # ACT queue order mirrors chain need: the c-path consumes
                # chunk 0 of i/g first, and tanh_c0 must not sit behind a
                # full-width sigmoid_o, so si/so are split in halves too.
                nc.scalar.activation(sf, gates[1], sigf)
                nc.scalar.activation(si[:, 0:HF], gates[0][:, 0:HF], sigf)
                nc.scalar.activation(tg[:, 0:HF], gates[3], tanhf)
                nc.scalar.activation(si[:, HF:512], gates[0][:, HF:512], sigf)
                nc.scalar.activation(tg[:, HF:512], gates[4], tanhf)
                nc.scalar.activation(so[:, 0:HF], gates[2][:, 0:HF], sigf)
                nc.vector.tensor_mul(fc, sf, c)
                # chunked: ig -> c -> tanh(c), halves pipelined so the next
                # MM stream can start once chunk 0 reaches hT below.
                for q in (0, 1):
                    s = slice(q * HF, (q + 1) * HF)
                    nc.vector.tensor_mul(ig[:, s], si[:, s], tg[:, s])
                    nc.vector.tensor_add(cn[:, s], ig[:, s], fc[:, s])
                nc.scalar.activation(tc_t[:, 0:HF], cn[:, 0:HF], tanhf)
                nc.scalar.activation(so[:, HF:512], gates[2][:, HF:512], sigf)
                nc.scalar.activation(tc_t[:, HF:512], cn[:, HF:512], tanhf)

                # PE tail: next step's PSUM init, gx fill, transposes
                if t + 1 < t_steps:
                    gates_next = alloc_gates(t + 1)
                    emit_selectors(t + 1, gates_next)
                else:
                    gates_next = None
                gx_part = j if (w + 1 < nw and j < 4) else None
                if gx_part is not None:
                    emit_gx_mms(w + 1, gx_part)

                # hT = transpose(so) * transpose(tanh_c): the elementwise
                # multiply happens in the transposed domain, cutting the
                # h-mul + hT-copy off the critical chain.
                hTn = hT_pool.tile([128, 4 * BL], f32r, tag="hT")
                soT = trps_pool.tile([128, 4 * BL], f32, tag="soT", name=f"soT_{t}")
                tcT2 = trps_pool.tile([128, 4 * BL], f32, tag="tcT", name=f"tcT_{t}")
                tcT = [tcT2[:, 0 : 2 * BL], tcT2[:, 2 * BL : 4 * BL]]
                soT_sb = ep_pool.tile([128, 4 * BL], f32, tag="soTsb")
                for q in (0, 1):
                    s2 = slice(q * 2 * BL, (q + 1) * 2 * BL)
                    for kk in (0, 1):
                        k = q * 2 + kk
                        nc.tensor.transpose(
                            soT[:, k * BL : (k + 1) * BL],
                            so[:, k * 128 : (k + 1) * 128],
                            eye32[:, :],
                        )
                        nc.tensor.transpose(
                            tcT[q][:, kk * BL : (kk + 1) * BL],
                            tc_t[:, k * 128 : (k + 1) * 128],
                            eye32[:, :],
                        )
                    nc.vector.tensor_copy(soT_sb[:, s2], soT[:, s2])
                    nc.vector.tensor_mul(hTn[:, s2], soT_sb[:, s2], tcT[q])
                nc.sync.dma_start(out=y_d[t], in_=hTn)
                if gx_part is not None:
                    emit_gx_add(w + 1, gx_part)
                if w + 1 < nw and j == 0 and w + 2 < nw:
                    load_xT(w + 2)

                c = cn
                hT = hTn
                gates = gates_next

    nc.compile()
    return nc


def _get_program(t_steps: int):
    if t_steps not in _COMPILED:
        _COMPILED[t_steps] = _build_program(t_steps)
    return _COMPILED[t_steps]


# gate permutation [i, f, o, g] from torch order [i, f, g, o]
_PERM = np.concatenate(
    [np.arange(0, 512), np.arange(512, 1024), np.arange(1536, 2048), np.arange(1024, 1536)]
)


def _host_prep(x, Wx, bx, Wh, bh, t_steps):
    WxT = np.ascontiguousarray(Wx[_PERM].T)
    WhT = np.ascontiguousarray(Wh[_PERM].T)
    b = (bx + bh)[_PERM].astype(np.float32)
    b128 = np.ascontiguousarray(np.broadcast_to(b, (128, G4)))
    eye = np.eye(128, dtype=np.float32)
    in_maps = []
    for c in range(8):
        d, g = divmod(c, 4)
        xc = x[g * BL : (g + 1) * BL, :t_steps]
        if d == 1:
            xc = xc[:, ::-1]
        xT = np.ascontiguousarray(xc.transpose(2, 1, 0).reshape(I, t_steps * BL))
        in_maps.append(
            {"xT": xT, "WxT": WxT, "WhT": WhT, "b128": b128, "eye": eye,
             "z": np.zeros((128, 4 * BL), np.float32),
             "eye32": np.eye(BL, dtype=np.float32)}
        )
    return in_maps


def kernel(x, Wx, bx, Wh, bh):
    from concourse.bass_utils import run_bass_kernel_spmd

    x = np.asarray(x, dtype=np.float32)
    Wx = np.asarray(Wx, dtype=np.float32)
    bx = np.asarray(bx, dtype=np.float32)
    Wh = np.asarray(Wh, dtype=np.float32)
    bh = np.asarray(bh, dtype=np.float32)
    nc = _get_program(T)
    in_maps = _host_prep(x, Wx, bx, Wh, bh, T)
    res = run_bass_kernel_spmd(nc, in_maps, list(range(8)))
    out = np.empty((B, T, 2 * H), dtype=np.float32)
    for c in range(8):
        d, g = divmod(c, 4)
        y = res.results[c]["y"]  # [T, 128, 4*BL] transposed-h layout
        yh = y.reshape(T, 128, 4, BL).transpose(0, 3, 2, 1).reshape(T, BL, H)
        out[g * BL : (g + 1) * BL, :, d * H : (d + 1) * H] = yh.transpose(1, 0, 2)
    return out


def _np_lstm(x, Wx, bx, Wh, bh):
    """Single-direction numpy reference for self-test (forward order)."""
    b_, t_, _ = x.shape
    h = np.zeros((b_, H), np.float32)
    c = np.zeros((b_, H), np.float32)
    gx = x @ Wx.T + bx
    ys = []
    for t in range(t_):
        gates = gx[:, t] + h @ Wh.T + bh
        i_g, f_g, g_g, o_g = np.split(gates, 4, axis=1)
        i_t = 1 / (1 + np.exp(-i_g))
        f_t = 1 / (1 + np.exp(-f_g))
        g_t = np.tanh(g_g)
        o_t = 1 / (1 + np.exp(-o_g))
        c = c * f_t + i_t * g_t
        h = o_t * np.tanh(c)
        ys.append(h)
    return np.stack(ys, 1)


def _selftest(t_steps=16, use_sim=True):
    from concourse.bass_interp import CoreSim

    rng = np.random.default_rng(0)
    s = 1.0 / np.sqrt(H)
    x = rng.standard_normal((B, T, I), dtype=np.float32)
    Wx = rng.standard_normal((G4, I), dtype=np.float32) * s
    bx = rng.standard_normal(G4).astype(np.float32) * s
    Wh = rng.standard_normal((G4, H), dtype=np.float32) * s
    bh = rng.standard_normal(G4).astype(np.float32) * s

    nc = _get_program(t_steps)
    in_maps = _host_prep(x, Wx, bx, Wh, bh, t_steps)
    sim = CoreSim(nc, trace=False)
    for k, v in in_maps[0].items():
        sim.tensor(k)[:] = v
    sim.simulate()
    y = np.array(sim.tensor("y"))  # [t, 128, 4*BL]
    yh = y.reshape(t_steps, 128, 4, BL).transpose(0, 3, 2, 1).reshape(t_steps, BL, H)
    ref = _np_lstm(x[:BL, :t_steps], Wx, bx, Wh, bh)  # [BL, t, H]
    err = np.abs(yh.transpose(1, 0, 2) - ref)
    scale = np.abs(ref).max()
    print(f"selftest T={t_steps}: max abs err {err.max():.3e} (scale {scale:.3f})")
    return err.max()


if __name__ == "__main__":
    _selftest(16)



# revision 4
# speedup vs baseline: 2.4717x; 2.4717x over previous
"""BiLSTM Trainium2 kernel — transposed (weight-stationary) design.

Problem: B=32, T=512, I=512, H=512 bidirectional LSTM (torch gate order
i,f,g,o; shared weights across directions; backward outputs stacked in
processing order).

Sharding: 8 cores = 2 directions x 4 batch groups of 8 rows. Every core runs
the IDENTICAL program; backward cores get time-reversed x (host prep).

Layout (the key idea): hidden dim lives on PARTITIONS, batch (8) is the
matmul moving dim. The 2048-wide gate dim is 16 chunks of 128 partitions,
chunk c -> (gate g=c//4, hidden-chunk q=c%4), gate order [i,f,g,o].

  - gx windows of W=4 steps are precomputed into PSUM bank tiles
    [128, 16 chunks, 32] (one full PSUM bank each), bias folded in via a
    K=1 ones-row matmul. The per-step recurrence h @ Wh.T accumulates
    INTO the same PSUM region (start=False), so there are no PSUM-init
    "selector" matmuls and no gx round trip.
  - Recurrent matmuls are weight-stationary: lhsT = WhT 128x128 chunk,
    rhs = h [128, 8]. 64 matmuls x 8 moving rows per step (bf16).
  - Sigmoid-only epilogue: g-gate weights are pre-scaled x2 on the host so
    tanh(g) = 2*sigmoid(2g)-1; the cell state is kept as c/2 and h as h/2
    (Wh pre-scaled x2 to compensate, y rescaled x2 on the host):
        fc = sigm(f) * c_half
        v  = (sigm(2g) - 0.5) * sigm(i)        [scalar_tensor_tensor]
        c_half' = v + fc
        sc = sigmoid(4 * c_half')               [activation scale=4]
        h_half = (sc - 0.5) * sigm(o)           [scalar_tensor_tensor]
    h_half is written in bf16 directly into the big y SBUF buffer, which
    doubles as the matmul rhs for the next step. One DMA stores y at the end.
"""

import numpy as np
from ml_dtypes import bfloat16

B, T, I, H = 32, 512, 512, 512
G4 = 4 * H          # 2048 gate width
BL = 8              # batch rows per core
W = 4               # steps per PSUM window bank
NCH = 16            # gate-dim chunks of 128
AHEAD = 4           # windows of gx lookahead

_COMPILED = {}


def _build_program(t_steps: int):
    import concourse.bass as bass
    import concourse.tile as tile
    from concourse import bacc, mybir

    dt = mybir.dt
    f32 = dt.float32
    bf16 = dt.bfloat16
    add = mybir.AluOpType.add
    mult = mybir.AluOpType.mult
    sigf = mybir.ActivationFunctionType.Sigmoid

    nw = t_steps // W

    nc = bacc.Bacc("TRN2", target_bir_lowering=False, debug=False)

    xT_d = nc.declare_dram_parameter("xT", [I, t_steps * BL], bf16, isOutput=False)
    whT_d = nc.declare_dram_parameter("whT", [H, G4], bf16, isOutput=False)
    wxT_d = nc.declare_dram_parameter("wxT", [I, G4], bf16, isOutput=False)
    b_d = nc.declare_dram_parameter("b1p", [1, G4], bf16, isOutput=False)
    ones_d = nc.declare_dram_parameter("ones", [1, W * BL], bf16, isOutput=False)
    y_d = nc.declare_dram_parameter("y", [128, t_steps, 4, BL], bf16, isOutput=True)

    with tile.TileContext(nc) as tc:
        with (
            tc.tile_pool(name="const", bufs=1) as cpool,
            tc.tile_pool(name="bank", bufs=6, space="PSUM") as bankpool,
            tc.tile_pool(name="ep", bufs=2) as ep,
            tc.tile_pool(name="cst", bufs=2) as cst,
        ):
            # ---- constants ----
            xT = []
            for k in range(4):
                t_ = cpool.tile([128, t_steps * BL], bf16, tag=f"xT{k}", name=f"xT{k}")
                nc.sync.dma_start(out=t_, in_=xT_d[k * 128 : (k + 1) * 128, :])
                xT.append(t_)
            whT = []
            for k in range(4):
                t_ = cpool.tile([128, G4], bf16, tag=f"whT{k}", name=f"whT{k}")
                nc.sync.dma_start(out=t_, in_=whT_d[k * 128 : (k + 1) * 128, :])
                whT.append(t_)
            wxT = []
            for k in range(4):
                t_ = cpool.tile([128, G4], bf16, tag=f"wxT{k}", name=f"wxT{k}")
                nc.sync.dma_start(out=t_, in_=wxT_d[k * 128 : (k + 1) * 128, :])
                wxT.append(t_)
            b1p = cpool.tile([1, G4], bf16, tag="b1p")
            nc.sync.dma_start(out=b1p, in_=b_d[:, :])
            ones = cpool.tile([1, W * BL], bf16, tag="ones")
            nc.sync.dma_start(out=ones, in_=ones_d[:, :])

            # y buffer: slot 0 is h_{-1} = 0, slot t+1 holds h_half(t) in bf16
            y_sb = cpool.tile([128, t_steps + 1, 4, BL], bf16, tag="y")
            nc.vector.memset(y_sb[:, 0], 0.0)
            c_prev = cst.tile([128, 4, BL], f32, tag="c", name="c_init")
            nc.vector.memset(c_prev, 0.0)

            # ---- gx windows ----
            banks = {}

            def emit_gx(w, cs):
                if w >= nw:
                    return
                first = w not in banks
                if first:
                    banks[w] = bankpool.tile(
                        [128, NCH, W * BL], f32, tag="bank", name=f"bank{w}"
                    )
                bk = banks[w]
                for c in cs:
                    for k in range(4):
                        # start=True marks the WHOLE psum bank pending-zero
                        # (lazy reset); exactly one per window bank.
                        nc.tensor.matmul(
                            bk[:, c, :],
                            lhsT=wxT[k][:, c * 128 : (c + 1) * 128],
                            rhs=xT[k][:, w * W * BL : (w + 1) * W * BL],
                            start=(first and k == 0 and c == cs[0]),
                            stop=False,
                            skip_group_check=True,
                        )
                    nc.tensor.matmul(
                        bk[:, c, :],
                        lhsT=b1p[0:1, c * 128 : (c + 1) * 128],
                        rhs=ones[0:1, :],
                        start=False,
                        stop=False,
                        skip_group_check=True,
                    )

            for w in range(AHEAD):
                emit_gx(w, range(NCH))

            # ---- main loop ----
            for t in range(t_steps):
                w, j = divmod(t, W)
                bk = banks[w]
                # future gx fills PE idle while waiting for h(t-1)
                emit_gx(w + AHEAD, range(4 * j, 4 * j + 4))

                jsl = slice(j * BL, (j + 1) * BL)
                for c in range(NCH):  # i,f,g chunks first, o last (layout order)
                    for k in range(4):
                        nc.tensor.matmul(
                            bk[:, c, jsl],
                            lhsT=whT[k][:, c * 128 : (c + 1) * 128],
                            rhs=y_sb[:, t, k, :],
                            start=False,
                            stop=(k == 3),
                            skip_group_check=True,
                        )

                s_ifg = ep.tile([128, 12, BL], f32, tag="sifg", name=f"sifg{t}")
                nc.scalar.activation(s_ifg, bk[:, 0:12, jsl], sigf)
                so = ep.tile([128, 4, BL], f32, tag="so", name=f"so{t}")
                nc.scalar.activation(so, bk[:, 12:16, jsl], sigf)

                fc = ep.tile([128, 4, BL], f32, tag="fc", name=f"fc{t}")
                nc.vector.tensor_mul(fc, s_ifg[:, 4:8, :], c_prev)
                v = ep.tile([128, 4, BL], f32, tag="v", name=f"v{t}")
                nc.vector.scalar_tensor_tensor(
                    v, s_ifg[:, 8:12, :], -0.5, s_ifg[:, 0:4, :], add, mult
                )
                c_new = cst.tile([128, 4, BL], f32, tag="c", name=f"c{t}")
                nc.vector.tensor_add(c_new, v, fc)
                sc = ep.tile([128, 4, BL], f32, tag="sc", name=f"sc{t}")
                nc.scalar.activation(sc, c_new, sigf, scale=4.0)
                nc.vector.scalar_tensor_tensor(
                    y_sb[:, t + 1], sc, -0.5, so, add, mult
                )
                c_prev = c_new
                if j == W - 1:
                    del banks[w]

            nc.sync.dma_start(out=y_d[:, :, :, :], in_=y_sb[:, 1:])

    nc.compile()
    return nc


def _get_program(t_steps: int):
    if t_steps not in _COMPILED:
        _COMPILED[t_steps] = _build_program(t_steps)
    return _COMPILED[t_steps]


def _host_prep(x, Wx, bx, Wh, bh, t_steps):
    # gate order already [i,f,g,o] (torch). Scales: h is carried as h/2 so
    # Wh cols x2; g rows additionally x2 (tanh via sigmoid) -> Wh g-cols x4,
    # Wx g-cols x2, bias g x2.
    wh_scale = np.full(G4, 2.0, np.float32)
    wh_scale[2 * H : 3 * H] = 4.0
    wx_scale = np.ones(G4, np.float32)
    wx_scale[2 * H : 3 * H] = 2.0
    whT = np.ascontiguousarray((Wh.T * wh_scale[None, :]).astype(bfloat16))
    wxT = np.ascontiguousarray((Wx.T * wx_scale[None, :]).astype(bfloat16))
    b = ((bx + bh) * wx_scale).astype(bfloat16).reshape(1, G4)
    ones = np.ones((1, W * BL), bfloat16)
    in_maps = []
    for core in range(8):
        d, g = divmod(core, 4)
        xc = x[g * BL : (g + 1) * BL, :t_steps]  # [8, T, I]
        if d == 1:
            xc = xc[:, ::-1]
        xTc = np.ascontiguousarray(
            xc.transpose(2, 1, 0).reshape(I, t_steps * BL).astype(bfloat16)
        )
        in_maps.append(
            {"xT": xTc, "whT": whT, "wxT": wxT, "b1p": b, "ones": ones}
        )
    return in_maps


def kernel(x, Wx, bx, Wh, bh):
    from concourse.bass_utils import run_bass_kernel_spmd

    x = np.asarray(x, dtype=np.float32)
    Wx = np.asarray(Wx, dtype=np.float32)
    bx = np.asarray(bx, dtype=np.float32)
    Wh = np.asarray(Wh, dtype=np.float32)
    bh = np.asarray(bh, dtype=np.float32)
    nc = _get_program(T)
    in_maps = _host_prep(x, Wx, bx, Wh, bh, T)
    res = run_bass_kernel_spmd(nc, in_maps, list(range(8)))
    out = np.empty((B, T, 2 * H), dtype=np.float32)
    for core in range(8):
        d, g = divmod(core, 4)
        y = np.asarray(res.results[core]["y"], dtype=np.float32)  # [128,T,4,8]
        # h[b, t, q*128+p] = 2 * y[p, t, q, b]
        yh = 2.0 * y.transpose(3, 1, 2, 0).reshape(BL, T, H)
        out[g * BL : (g + 1) * BL, :, d * H : (d + 1) * H] = yh
    return out


def _np_lstm(x, Wx, bx, Wh, bh):
    """Single-direction numpy reference for self-test (forward order)."""
    b_, t_, _ = x.shape
    h = np.zeros((b_, H), np.float32)
    c = np.zeros((b_, H), np.float32)
    gx = x @ Wx.T + bx
    ys = []
    for t in range(t_):
        gates = gx[:, t] + h @ Wh.T + bh
        i_g, f_g, g_g, o_g = np.split(gates, 4, axis=1)
        i_t = 1 / (1 + np.exp(-i_g))
        f_t = 1 / (1 + np.exp(-f_g))
        g_t = np.tanh(g_g)
        o_t = 1 / (1 + np.exp(-o_g))
        c = c * f_t + i_t * g_t
        h = o_t * np.tanh(c)
        ys.append(h)
    return np.stack(ys, 1)


def _selftest(t_steps=16):
    from concourse.bass_interp import CoreSim

    rng = np.random.default_rng(0)
    s = 1.0 / np.sqrt(H)
    x = rng.standard_normal((B, T, I), dtype=np.float32)
    Wx = (rng.standard_normal((G4, I), dtype=np.float32) * s).astype(np.float32)
    bx = (rng.standard_normal(G4) * s).astype(np.float32)
    Wh = (rng.standard_normal((G4, H), dtype=np.float32) * s).astype(np.float32)
    bh = (rng.standard_normal(G4) * s).astype(np.float32)

    nc = _get_program(t_steps)
    in_maps = _host_prep(x, Wx, bx, Wh, bh, t_steps)
    sim = CoreSim(nc, trace=False)
    for k, v in in_maps[0].items():
        sim.tensor(k)[:] = v
    sim.simulate()
    y = np.asarray(sim.tensor("y"), dtype=np.float32)  # [128, t, 4, 8]
    yh = 2.0 * y.transpose(3, 1, 2, 0).reshape(BL, t_steps, H)
    ref = _np_lstm(x[:BL, :t_steps], Wx, bx, Wh, bh)  # [BL, t, H]
    err = np.abs(yh - ref)
    scale = np.abs(ref).max()
    print(f"selftest T={t_steps}: max abs err {err.max():.3e} (scale {scale:.3f})")
    return err.max()


if __name__ == "__main__":
    _selftest(16)
